# revision 1
# baseline (speedup 1.0000x reference)
"""KGNN head kernel for Trainium2 (Bass/Tile), 8-core data-parallel SPMD.

Computation (per batch b):
    score[g, n] = sum_d drug[b, g, d] * rel[b, 8g+n, d]         (n in 0..8)
    out[b, g, d] = sum_n score[g, n] * ent[b, 8g+n, d]

Layout: one SBUF partition holds one (batch-pair-slot, group) row; the 8
neighbors x 64 dims of that group lie contiguously in the free dimension, so
every DMA is a straight contiguous stream (2KB runs per partition).

Per 64-element chunk:
  - DVE tensor_tensor_reduce fuses the (rel * drug) multiply with the dot
    over d -> per-partition score scalar.
  - tensor_scalar (DVE) / activation-with-scale (ACT) scales the matching
    ent chunk by the score.
  - The sum over the 8 neighbor chunks runs on the idle TensorEngine as
    PSUM-accumulating matmuls with a constant 128x128 identity as lhsT.
"""

import numpy as np

import concourse.bass as bass  # noqa: F401  (engine namespaces via nc)
import concourse.mybir as mybir
import concourse.tile as tile
from concourse import bacc
from concourse.bass_utils import run_bass_kernel_spmd
from concourse.masks import make_identity

F32 = mybir.dt.float32

N_CORES = 8
B_FULL = 2048
B_LOCAL = B_FULL // N_CORES  # 256
G = 64          # groups per sample
NN = 8          # neighbors per group
D = 64          # feature dim
S = G * NN      # 512 neighbor slots

SB = 16                 # batches per superblock
U = SB // 2             # 2-batch units per superblock (8)
N_SBLK = B_LOCAL // SB  # superblocks per core (16)

# chunks (of 8) whose ent-scaling runs on DVE; the rest go to ACT
DVE_W_CHUNKS = 2


def _build_nc(b_local: int = B_LOCAL) -> "bacc.Bacc":
    n_sblk = b_local // SB
    assert n_sblk * SB == b_local

    nc = bacc.Bacc("TRN2", target_bir_lowering=False, debug=False)

    drug_d = nc.dram_tensor("drug", [b_local, G, D], F32, kind="ExternalInput")
    rel_d = nc.dram_tensor("rel", [b_local, S, D], F32, kind="ExternalInput")
    ent_d = nc.dram_tensor("ent", [b_local, S, D], F32, kind="ExternalInput")
    out_d = nc.dram_tensor("out", [b_local, G, D], F32, kind="ExternalOutput")

    # partition p = (bb g); free = [u][(n d)] / [u][d]
    rel_v = rel_d[:].rearrange(
        "(s u bb) (g n) d -> s (bb g) u (n d)", s=n_sblk, u=U, bb=2, g=G, n=NN
    )
    ent_v = ent_d[:].rearrange(
        "(s u bb) (g n) d -> s (bb g) u (n d)", s=n_sblk, u=U, bb=2, g=G, n=NN
    )
    drug_v = drug_d[:].rearrange(
        "(s u bb) g d -> s (bb g) u d", s=n_sblk, u=U, bb=2
    )
    out_v = out_d[:].rearrange(
        "(s u bb) g d -> s (bb g) u d", s=n_sblk, u=U, bb=2
    )

    with tile.TileContext(nc) as tc:
        with (
            tc.tile_pool(name="const", bufs=1) as const_pool,
            tc.tile_pool(name="rel", bufs=2) as rel_pool,
            tc.tile_pool(name="ent", bufs=2) as ent_pool,
            tc.tile_pool(name="drug", bufs=2) as drug_pool,
            tc.tile_pool(name="score", bufs=3) as score_pool,
            tc.tile_pool(name="prod", bufs=2) as prod_pool,
            tc.tile_pool(name="w", bufs=3) as w_pool,
            tc.tile_pool(name="outs", bufs=2) as out_pool,
            tc.tile_pool(name="psum", bufs=2, space="PSUM") as psum_pool,
        ):
            ident = const_pool.tile([128, 128], F32)
            make_identity(nc, ident[:])

            for sb in range(n_sblk):
                rel_t = rel_pool.tile([128, U * NN * D], F32)
                nc.sync.dma_start(
                    out=rel_t[:].rearrange("p (u nd) -> p u nd", u=U), in_=rel_v[sb]
                )
                ent_t = ent_pool.tile([128, U * NN * D], F32)
                nc.sync.dma_start(
                    out=ent_t[:].rearrange("p (u nd) -> p u nd", u=U), in_=ent_v[sb]
                )
                drug_t = drug_pool.tile([128, U * D], F32)
                nc.sync.dma_start(
                    out=drug_t[:].rearrange("p (u d) -> p u d", u=U), in_=drug_v[sb]
                )

                # scores: one broadcast multiply + one segmented reduce
                # (tensor_tensor_reduce is broken on the HW path, probed)
                prod_t = prod_pool.tile([128, U * NN * D], F32)
                nc.vector.tensor_tensor(
                    out=prod_t[:].rearrange("p (u n d) -> p u n d", u=U, n=NN),
                    in0=rel_t[:].rearrange("p (u n d) -> p u n d", u=U, n=NN),
                    in1=drug_t[:]
                    .rearrange("p (u n d) -> p u n d", u=U, n=1)
                    .to_broadcast([128, U, NN, D]),
                    op=mybir.AluOpType.mult,
                )
                score_t = score_pool.tile([128, U * NN], F32)
                nc.vector.tensor_reduce(
                    out=score_t[:],
                    in_=prod_t[:].rearrange("p (un d) -> p un d", d=D),
                    axis=mybir.AxisListType.X,
                    op=mybir.AluOpType.add,
                )

                # weighted ent chunks, accumulated over n on the TensorEngine
                psum_t = psum_pool.tile([128, U * D], F32)
                for c in range(NN):
                    w_t = w_pool.tile([128, U * D], F32)
                    for u in range(U):
                        off = u * NN * D + c * D
                        src = ent_t[:, off : off + D]
                        dst = w_t[:, u * D : (u + 1) * D]
                        sc_ap = score_t[:, u * NN + c : u * NN + c + 1]
                        if c < DVE_W_CHUNKS:
                            nc.vector.tensor_scalar_mul(dst, src, sc_ap)
                        else:
                            nc.scalar.mul(dst, src, sc_ap)
                    nc.tensor.matmul(
                        out=psum_t[:],
                        lhsT=ident[:],
                        rhs=w_t[:],
                        start=(c == 0),
                        stop=(c == NN - 1),
                    )

                out_t = out_pool.tile([128, U * D], F32)
                nc.scalar.copy(out=out_t[:], in_=psum_t[:])
                nc.sync.dma_start(
                    out=out_v[sb],
                    in_=out_t[:].rearrange("p (u d) -> p u d", u=U),
                )

    nc.compile()
    return nc


_NC_CACHE: dict = {}


def _get_nc(b_local: int = B_LOCAL):
    if b_local not in _NC_CACHE:
        _NC_CACHE[b_local] = _build_nc(b_local)
    return _NC_CACHE[b_local]


def run_sharded(drug, rel, ent, trace: bool = False):
    """Shard batch dim across the 8 cores, run, gather. Returns
    (full output [B, G, D], BassKernelResults)."""
    drug = np.ascontiguousarray(np.asarray(drug, dtype=np.float32))
    rel = np.ascontiguousarray(np.asarray(rel, dtype=np.float32))
    ent = np.ascontiguousarray(np.asarray(ent, dtype=np.float32))
    b = drug.shape[0]
    nb = b // N_CORES
    assert nb * N_CORES == b
    nc = _get_nc(nb)
    in_maps = [
        {
            "drug": np.ascontiguousarray(drug[i * nb : (i + 1) * nb]),
            "rel": np.ascontiguousarray(rel[i * nb : (i + 1) * nb]),
            "ent": np.ascontiguousarray(ent[i * nb : (i + 1) * nb]),
        }
        for i in range(N_CORES)
    ]
    last_exc = None
    for attempt in range(3):
        try:
            res = run_bass_kernel_spmd(nc, in_maps, list(range(N_CORES)), trace=trace)
            break
        except Exception as exc:  # transient device-unrecoverable states
            last_exc = exc
            import time

            time.sleep(10 * (attempt + 1))
    else:
        raise last_exc
    out = np.concatenate([res.results[i]["out"] for i in range(N_CORES)], axis=0)
    return out, res


def kernel(drug, rel, ent):
    out, _ = run_sharded(drug, rel, ent, trace=False)
    return out



# revision 4
# speedup vs baseline: 1.3701x; 1.3701x over previous
"""KGNN head kernel for Trainium2 (Bass/Tile), 8-core data-parallel SPMD.

Computation (per batch b):
    score[g, n] = sum_d drug[b, g, d] * rel[b, 8g+n, d]         (n in 0..8)
    out[b, g, d] = sum_n score[g, n] * ent[b, 8g+n, d]

v2 design notes (vs the 356us v1):
  - Layout: partition p = (bb, g2) = 4 batch slots x 32 group-pairs, so each
    partition's DMA run is (gl n d) = 1024 floats = 4KB contiguous (v1 had 2KB
    runs; 4KB descriptors lift per-SDMA-engine rate toward line rate).
  - Scores: one fp32 tensor_tensor (drug broadcast over n) -> bf16 product,
    then a bf16 add-tree over d (2x DVE mode) instead of tensor_reduce (1x).
  - Scaling: ONE tensor_tensor per superblock with score broadcast over d
    (v1 burned 74% of ScalarE on 784 tiny 64-element ACTIVATEs).
  - Neighbor sum: 8 PSUM-accumulating matmuls vs a bf16 identity (v1 used
    fp32 matmuls -> hi/lo split = 2x MATMUL + 2x LDWEIGHTS traffic).
"""

import numpy as np

import concourse.bass as bass  # noqa: F401  (engine namespaces via nc)
import concourse.mybir as mybir
import concourse.tile as tile
from concourse import bacc
from concourse.bass_utils import run_bass_kernel_spmd
from concourse.masks import make_identity

F32 = mybir.dt.float32
BF16 = mybir.dt.bfloat16

N_CORES = 8
B_FULL = 2048
B_LOCAL = B_FULL // N_CORES  # 256
G = 64          # groups per sample
NN = 8          # neighbors per group
D = 64          # feature dim
S = G * NN      # 512 neighbor slots

BB = 4                  # batch slots per partition
G2 = 32                 # group-pairs per partition  (BB*G2 = 128 partitions)
GL = 2                  # groups per pair
SB = 16                 # batches per superblock
U = SB // BB            # batch units per superblock (4)
UG = U * GL             # (u, gl) combined (8)
RUN = GL * NN * D       # contiguous floats per (partition, u): 1024 (4KB)
SEG = UG * NN           # dot-product segments per partition (64)


def _build_nc(b_local: int = B_LOCAL) -> "bacc.Bacc":
    n_sblk = b_local // SB
    assert n_sblk * SB == b_local

    nc = bacc.Bacc("TRN2", target_bir_lowering=False, debug=False)

    drug_d = nc.dram_tensor("drug", [b_local, G, D], F32, kind="ExternalInput")
    rel_d = nc.dram_tensor("rel", [b_local, S, D], F32, kind="ExternalInput")
    ent_d = nc.dram_tensor("ent", [b_local, S, D], F32, kind="ExternalInput")
    out_d = nc.dram_tensor("out", [b_local, G, D], F32, kind="ExternalOutput")

    # partition p = (bb g2); batch = s*SB + u*BB + bb; group = g2*GL + gl
    rel_v = rel_d[:].rearrange(
        "(s u bb) (g2 gl n) d -> s (bb g2) u (gl n d)",
        s=n_sblk, u=U, bb=BB, g2=G2, gl=GL, n=NN,
    )
    ent_v = ent_d[:].rearrange(
        "(s u bb) (g2 gl n) d -> s (bb g2) u (gl n d)",
        s=n_sblk, u=U, bb=BB, g2=G2, gl=GL, n=NN,
    )
    drug_v = drug_d[:].rearrange(
        "(s u bb) (g2 gl) d -> s (bb g2) u (gl d)",
        s=n_sblk, u=U, bb=BB, g2=G2, gl=GL,
    )
    out_v = out_d[:].rearrange(
        "(s u bb) (g2 gl) d -> s (bb g2) u (gl d)",
        s=n_sblk, u=U, bb=BB, g2=G2, gl=GL,
    )

    with tile.TileContext(nc) as tc:
        with (
            tc.tile_pool(name="const", bufs=1) as const_pool,
            tc.tile_pool(name="rel", bufs=2) as rel_pool,
            tc.tile_pool(name="ent", bufs=2) as ent_pool,
            tc.tile_pool(name="drug", bufs=2) as drug_pool,
            tc.tile_pool(name="prod", bufs=2) as prod_pool,
            tc.tile_pool(name="t32", bufs=2) as t32_pool,
            tc.tile_pool(name="t16", bufs=2) as t16_pool,
            tc.tile_pool(name="t8", bufs=2) as t8_pool,
            tc.tile_pool(name="t4", bufs=2) as t4_pool,
            tc.tile_pool(name="t2", bufs=2) as t2_pool,
            tc.tile_pool(name="score", bufs=2) as score_pool,
            tc.tile_pool(name="w", bufs=2) as w_pool,
            tc.tile_pool(name="outs", bufs=2) as out_pool,
            tc.tile_pool(name="psum", bufs=2, space="PSUM") as psum_pool,
        ):
            ident = const_pool.tile([128, 128], BF16)
            make_identity(nc, ident[:])

            for sb in range(n_sblk):
                rel_t = rel_pool.tile([128, U * RUN], F32)
                nc.sync.dma_start(
                    out=rel_t[:].rearrange("p (u r) -> p u r", u=U), in_=rel_v[sb]
                )
                ent_t = ent_pool.tile([128, U * RUN], F32)
                nc.sync.dma_start(
                    out=ent_t[:].rearrange("p (u r) -> p u r", u=U), in_=ent_v[sb]
                )
                drug_t = drug_pool.tile([128, U * GL * D], F32)
                nc.sync.dma_start(
                    out=drug_t[:].rearrange("p (u r) -> p u r", u=U), in_=drug_v[sb]
                )

                # product (rel * drug), drug broadcast over the neighbor axis
                prod_t = prod_pool.tile([128, U * RUN], BF16)
                nc.vector.tensor_tensor(
                    out=prod_t[:].rearrange("p (ug n d) -> p ug n d", ug=UG, n=NN),
                    in0=rel_t[:].rearrange("p (ug n d) -> p ug n d", ug=UG, n=NN),
                    in1=drug_t[:]
                    .rearrange("p (ug n d) -> p ug n d", ug=UG, n=1)
                    .to_broadcast([128, UG, NN, D]),
                    op=mybir.AluOpType.mult,
                )

                # dot over d: bf16 halving tree (2x DVE mode per level)
                cur = prod_t
                width = D
                for pool in (t32_pool, t16_pool, t8_pool, t4_pool, t2_pool):
                    width //= 2
                    nxt = pool.tile([128, SEG * width], BF16)
                    nc.vector.tensor_tensor(
                        out=nxt[:].rearrange("p (s w) -> p s w", s=SEG),
                        in0=cur[:].rearrange("p (s w) -> p s w", s=SEG, w=2 * width)[
                            :, :, 0:width
                        ],
                        in1=cur[:].rearrange("p (s w) -> p s w", s=SEG, w=2 * width)[
                            :, :, width : 2 * width
                        ],
                        op=mybir.AluOpType.add,
                    )
                    cur = nxt
                score_t = score_pool.tile([128, SEG], BF16)
                nc.vector.tensor_tensor(
                    out=score_t[:].rearrange("p (s w) -> p s w", s=SEG),
                    in0=cur[:].rearrange("p (s w) -> p s w", s=SEG, w=2)[:, :, 0:1],
                    in1=cur[:].rearrange("p (s w) -> p s w", s=SEG, w=2)[:, :, 1:2],
                    op=mybir.AluOpType.add,
                )

                # weighted ent; w laid out [n][ug][d] so each neighbor chunk is
                # a contiguous 512-col matmul rhs
                w_t = w_pool.tile([128, U * RUN], BF16)
                nc.vector.tensor_tensor(
                    out=w_t[:].rearrange("p (n ug d) -> p ug n d", n=NN, ug=UG),
                    in0=ent_t[:].rearrange("p (ug n d) -> p ug n d", ug=UG, n=NN),
                    in1=score_t[:]
                    .rearrange("p (ug n d) -> p ug n d", ug=UG, n=NN, d=1)
                    .to_broadcast([128, UG, NN, D]),
                    op=mybir.AluOpType.mult,
                )

                # neighbor sum on the TensorEngine: psum += I.T @ w[n]
                psum_t = psum_pool.tile([128, UG * D], F32)
                for c in range(NN):
                    nc.tensor.matmul(
                        out=psum_t[:],
                        lhsT=ident[:],
                        rhs=w_t[:, c * UG * D : (c + 1) * UG * D],
                        start=(c == 0),
                        stop=(c == NN - 1),
                    )

                out_t = out_pool.tile([128, UG * D], F32)
                nc.scalar.copy(out=out_t[:], in_=psum_t[:])
                nc.sync.dma_start(
                    out=out_v[sb],
                    in_=out_t[:].rearrange("p (u r) -> p u r", u=U),
                )

    nc.compile()
    return nc


_NC_CACHE: dict = {}


def _get_nc(b_local: int = B_LOCAL):
    if b_local not in _NC_CACHE:
        _NC_CACHE[b_local] = _build_nc(b_local)
    return _NC_CACHE[b_local]


def run_sharded(drug, rel, ent, trace: bool = False):
    """Shard batch dim across the 8 cores, run, gather. Returns
    (full output [B, G, D], BassKernelResults)."""
    drug = np.ascontiguousarray(np.asarray(drug, dtype=np.float32))
    rel = np.ascontiguousarray(np.asarray(rel, dtype=np.float32))
    ent = np.ascontiguousarray(np.asarray(ent, dtype=np.float32))
    b = drug.shape[0]
    nb = b // N_CORES
    assert nb * N_CORES == b
    nc = _get_nc(nb)
    in_maps = [
        {
            "drug": np.ascontiguousarray(drug[i * nb : (i + 1) * nb]),
            "rel": np.ascontiguousarray(rel[i * nb : (i + 1) * nb]),
            "ent": np.ascontiguousarray(ent[i * nb : (i + 1) * nb]),
        }
        for i in range(N_CORES)
    ]
    last_exc = None
    for attempt in range(3):
        try:
            res = run_bass_kernel_spmd(nc, in_maps, list(range(N_CORES)), trace=trace)
            break
        except Exception as exc:  # transient device-unrecoverable states
            last_exc = exc
            import time

            time.sleep(10 * (attempt + 1))
    else:
        raise last_exc
    out = np.concatenate([res.results[i]["out"] for i in range(N_CORES)], axis=0)
    return out, res


def kernel(drug, rel, ent):
    out, _ = run_sharded(drug, rel, ent, trace=False)
    return out


# revision 5
# speedup vs baseline: 1.7123x; 1.2498x over previous
"""KGNN head kernel for Trainium2 (Bass/Tile), 8-core data-parallel SPMD.

Computation (per batch b):
    score[g, n] = sum_d drug[b, g, d] * rel[b, 8g+n, d]         (n in 0..8)
    out[b, g, d] = sum_n score[g, n] * ent[b, 8g+n, d]

v2 design notes (vs the 356us v1):
  - Layout: partition p = (bb, g2) = 4 batch slots x 32 group-pairs, so each
    partition's DMA run is (gl n d) = 1024 floats = 4KB contiguous (v1 had 2KB
    runs; 4KB descriptors lift per-SDMA-engine rate toward line rate).
  - Scores: one fp32 tensor_tensor (drug broadcast over n) -> bf16 product,
    then a bf16 add-tree over d (2x DVE mode) instead of tensor_reduce (1x).
  - Scaling: ONE tensor_tensor per superblock with score broadcast over d
    (v1 burned 74% of ScalarE on 784 tiny 64-element ACTIVATEs).
  - Neighbor sum: 8 PSUM-accumulating matmuls vs a bf16 identity (v1 used
    fp32 matmuls -> hi/lo split = 2x MATMUL + 2x LDWEIGHTS traffic).
"""

import numpy as np

import concourse.bass as bass  # noqa: F401  (engine namespaces via nc)
import concourse.mybir as mybir
import concourse.tile as tile
from concourse import bacc
from concourse.bass_utils import run_bass_kernel_spmd
from concourse.masks import make_identity

F32 = mybir.dt.float32
BF16 = mybir.dt.bfloat16

N_CORES = 8
B_FULL = 2048
B_LOCAL = B_FULL // N_CORES  # 256
G = 64          # groups per sample
NN = 8          # neighbors per group
D = 64          # feature dim
S = G * NN      # 512 neighbor slots

BB = 8                  # batch slots per partition
G2 = 16                 # group-quads per partition  (BB*G2 = 128 partitions)
GL = 4                  # groups per quad
SB = 16                 # batches per superblock
U = SB // BB            # batch units per superblock (2)
UG = U * GL             # (u, gl) combined (8)
RUN = GL * NN * D       # contiguous floats per (partition, u): 1024 (4KB)
SEG = UG * NN           # dot-product segments per partition (64)


def _build_nc(b_local: int = B_LOCAL) -> "bacc.Bacc":
    n_sblk = b_local // SB
    assert n_sblk * SB == b_local

    nc = bacc.Bacc("TRN2", target_bir_lowering=False, debug=False)

    drug_d = nc.dram_tensor("drug", [b_local, G, D], F32, kind="ExternalInput")
    rel_d = nc.dram_tensor("rel", [b_local, S, D], F32, kind="ExternalInput")
    ent_d = nc.dram_tensor("ent", [b_local, S, D], F32, kind="ExternalInput")
    out_d = nc.dram_tensor("out", [b_local, G, D], F32, kind="ExternalOutput")

    # partition p = (bb g2); batch = s*SB + u*BB + bb; group = g2*GL + gl
    rel_v = rel_d[:].rearrange(
        "(s u bb) (g2 gl n) d -> s (bb g2) u (gl n d)",
        s=n_sblk, u=U, bb=BB, g2=G2, gl=GL, n=NN,
    )
    ent_v = ent_d[:].rearrange(
        "(s u bb) (g2 gl n) d -> s (bb g2) u (gl n d)",
        s=n_sblk, u=U, bb=BB, g2=G2, gl=GL, n=NN,
    )
    drug_v = drug_d[:].rearrange(
        "(s u bb) (g2 gl) d -> s (bb g2) u (gl d)",
        s=n_sblk, u=U, bb=BB, g2=G2, gl=GL,
    )
    out_v = out_d[:].rearrange(
        "(s u bb) (g2 gl) d -> s (bb g2) u (gl d)",
        s=n_sblk, u=U, bb=BB, g2=G2, gl=GL,
    )

    with tile.TileContext(nc) as tc:
        with (
            tc.tile_pool(name="const", bufs=1) as const_pool,
            tc.tile_pool(name="rel", bufs=2) as rel_pool,
            tc.tile_pool(name="ent", bufs=2) as ent_pool,
            tc.tile_pool(name="drug", bufs=2) as drug_pool,
            tc.tile_pool(name="prod", bufs=2) as prod_pool,
            tc.tile_pool(name="t32", bufs=2) as t32_pool,
            tc.tile_pool(name="t16", bufs=2) as t16_pool,
            tc.tile_pool(name="t8", bufs=2) as t8_pool,
            tc.tile_pool(name="t4", bufs=2) as t4_pool,
            tc.tile_pool(name="t2", bufs=2) as t2_pool,
            tc.tile_pool(name="score", bufs=2) as score_pool,
            tc.tile_pool(name="w", bufs=2) as w_pool,
            tc.tile_pool(name="outs", bufs=2) as out_pool,
            tc.tile_pool(name="psum", bufs=2, space="PSUM") as psum_pool,
        ):
            ident = const_pool.tile([128, 128], BF16)
            make_identity(nc, ident[:])

            for sb in range(n_sblk):
                rel_t = rel_pool.tile([128, U * RUN], BF16)
                nc.gpsimd.dma_start(
                    out=rel_t[:].rearrange("p (u r) -> p u r", u=U), in_=rel_v[sb]
                )
                ent_t = ent_pool.tile([128, U * RUN], F32)
                nc.sync.dma_start(
                    out=ent_t[:].rearrange("p (u r) -> p u r", u=U), in_=ent_v[sb]
                )
                drug_t = drug_pool.tile([128, U * GL * D], BF16)
                nc.gpsimd.dma_start(
                    out=drug_t[:].rearrange("p (u r) -> p u r", u=U), in_=drug_v[sb]
                )

                # product (rel * drug), drug broadcast over the neighbor axis
                prod_t = prod_pool.tile([128, U * RUN], BF16)
                nc.vector.tensor_tensor(
                    out=prod_t[:].rearrange("p (ug n d) -> p ug n d", ug=UG, n=NN),
                    in0=rel_t[:].rearrange("p (ug n d) -> p ug n d", ug=UG, n=NN),
                    in1=drug_t[:]
                    .rearrange("p (ug n d) -> p ug n d", ug=UG, n=1)
                    .to_broadcast([128, UG, NN, D]),
                    op=mybir.AluOpType.mult,
                )

                # dot over d: bf16 halving tree (2x DVE mode per level)
                cur = prod_t
                width = D
                for pool in (t32_pool, t16_pool, t8_pool, t4_pool, t2_pool):
                    width //= 2
                    nxt = pool.tile([128, SEG * width], BF16)
                    nc.vector.tensor_tensor(
                        out=nxt[:].rearrange("p (s w) -> p s w", s=SEG),
                        in0=cur[:].rearrange("p (s w) -> p s w", s=SEG, w=2 * width)[
                            :, :, 0:width
                        ],
                        in1=cur[:].rearrange("p (s w) -> p s w", s=SEG, w=2 * width)[
                            :, :, width : 2 * width
                        ],
                        op=mybir.AluOpType.add,
                    )
                    cur = nxt
                score_t = score_pool.tile([128, SEG], BF16)
                nc.vector.tensor_tensor(
                    out=score_t[:].rearrange("p (s w) -> p s w", s=SEG),
                    in0=cur[:].rearrange("p (s w) -> p s w", s=SEG, w=2)[:, :, 0:1],
                    in1=cur[:].rearrange("p (s w) -> p s w", s=SEG, w=2)[:, :, 1:2],
                    op=mybir.AluOpType.add,
                )

                # weighted ent; w laid out [n][ug][d] so each neighbor chunk is
                # a contiguous 512-col matmul rhs
                w_t = w_pool.tile([128, U * RUN], BF16)
                nc.vector.tensor_tensor(
                    out=w_t[:].rearrange("p (n ug d) -> p ug n d", n=NN, ug=UG),
                    in0=ent_t[:].rearrange("p (ug n d) -> p ug n d", ug=UG, n=NN),
                    in1=score_t[:]
                    .rearrange("p (ug n d) -> p ug n d", ug=UG, n=NN, d=1)
                    .to_broadcast([128, UG, NN, D]),
                    op=mybir.AluOpType.mult,
                )

                # neighbor sum on the TensorEngine: psum += I.T @ w[n]
                psum_t = psum_pool.tile([128, UG * D], F32)
                for c in range(NN):
                    nc.tensor.matmul(
                        out=psum_t[:],
                        lhsT=ident[:],
                        rhs=w_t[:, c * UG * D : (c + 1) * UG * D],
                        start=(c == 0),
                        stop=(c == NN - 1),
                    )

                out_t = out_pool.tile([128, UG * D], F32)
                nc.scalar.copy(out=out_t[:], in_=psum_t[:])
                nc.sync.dma_start(
                    out=out_v[sb],
                    in_=out_t[:].rearrange("p (u r) -> p u r", u=U),
                )

    nc.compile()
    return nc


_NC_CACHE: dict = {}


def _get_nc(b_local: int = B_LOCAL):
    if b_local not in _NC_CACHE:
        _NC_CACHE[b_local] = _build_nc(b_local)
    return _NC_CACHE[b_local]


def run_sharded(drug, rel, ent, trace: bool = False):
    """Shard batch dim across the 8 cores, run, gather. Returns
    (full output [B, G, D], BassKernelResults)."""
    drug = np.ascontiguousarray(np.asarray(drug, dtype=np.float32))
    rel = np.ascontiguousarray(np.asarray(rel, dtype=np.float32))
    ent = np.ascontiguousarray(np.asarray(ent, dtype=np.float32))
    b = drug.shape[0]
    nb = b // N_CORES
    assert nb * N_CORES == b
    nc = _get_nc(nb)
    in_maps = [
        {
            "drug": np.ascontiguousarray(drug[i * nb : (i + 1) * nb]),
            "rel": np.ascontiguousarray(rel[i * nb : (i + 1) * nb]),
            "ent": np.ascontiguousarray(ent[i * nb : (i + 1) * nb]),
        }
        for i in range(N_CORES)
    ]
    last_exc = None
    for attempt in range(3):
        try:
            res = run_bass_kernel_spmd(nc, in_maps, list(range(N_CORES)), trace=trace)
            break
        except Exception as exc:  # transient device-unrecoverable states
            last_exc = exc
            import time

            time.sleep(10 * (attempt + 1))
    else:
        raise last_exc
    out = np.concatenate([res.results[i]["out"] for i in range(N_CORES)], axis=0)
    return out, res


def kernel(drug, rel, ent):
    out, _ = run_sharded(drug, rel, ent, trace=False)
    return out
